# revision 16
# baseline (speedup 1.0000x reference)
"""BPTRU cell kernel for Trainium2, 8 NeuronCores, batch-sharded.

Reference computation (per batch row b, sequence length L):
  xh = x @ W_u.T                                   # [L, 4H] projections
  per step: x1,x2,x3,x4 = split(xh[t])
    f = sigmoid(x1 + v_f*c + b_f)
    r = sigmoid(x2 + v_r*c + b_r)
    o = sigmoid(c @ W_c.T + h @ W_h.T + b_o)
    c' = tanh(f*c + (1-f)*x3 + o*h)
    h' = tanh(r*h + (1-r)*x4 + o*c)
  returns (outs [B,L,H], h_T, c_T)

Sharding: batch 64 -> 8 per core, weights replicated (no cross-core comm).
Per-core layouts:
  - projection: out = W_u.T.T @ x.T via PE, float32r (TF32-ish), results
    stored to DRAM as [t, gate, m, b, p] so per-step loads are contiguous.
  - recurrence state S [128, 128] fp32: cols 0:64 = c, 64:128 = h, packed
    (p, m*8+b) with H index = m*128+p.  Matmul weight-stationary bf16.
"""

import os
import numpy as np
import ml_dtypes

import concourse.bass as bass
import concourse.mybir as mybir
import concourse.tile as tile
from concourse import bacc, bass_utils

B, L, I, H = 64, 512, 512, 1024
NCORES = 8
BL = B // NCORES  # 8 batch rows per core
GH = H // 128     # 8 h-chunks of 128

L_STEPS = int(os.environ.get("BPTRU_L", L))
REC_DTYPE = os.environ.get("BPTRU_REC_DTYPE", "f32r")  # bf16 | f32r
REPEAT = int(os.environ.get("BPTRU_REPEAT", 1))

_CACHE = {}


def _build_nc(l_steps: int, rec_dtype: str, repeat: int = 1):
    nc = bacc.Bacc("TRN2", target_bir_lowering=False, debug=False,
                   num_devices=NCORES)
    f32 = mybir.dt.float32
    f32r = mybir.dt.float32r
    bf16 = mybir.dt.bfloat16
    wdt = bf16 if rec_dtype == "bf16" else f32r
    sdt = f32 if rec_dtype == "bf16" else f32r

    # ---- inputs (per-core) ----
    xT = nc.dram_tensor("xT", [I, BL * L], f32r, kind="ExternalInput")
    wuT = nc.dram_tensor("wuT", [I, 4 * H], f32r, kind="ExternalInput")
    # [W_c.T ; W_h.T] stacked -> lhsT for the recurrence matmul
    wT = nc.dram_tensor("wT", [2 * H, H], wdt, kind="ExternalInput")
    vf_t = nc.dram_tensor("vf_t", [128, 64], f32, kind="ExternalInput")
    vr_t = nc.dram_tensor("vr_t", [128, 64], f32, kind="ExternalInput")
    bo_t = nc.dram_tensor("bo_t", [128, 64], f32, kind="ExternalInput")
    bfr = nc.dram_tensor("bfr", [128, 16], f32, kind="ExternalInput")

    # ---- outputs (per-core); outs layout [t, m, b, p], host transposes ----
    outs = nc.dram_tensor("outs", [l_steps, GH, BL, 128], sdt,
                          kind="ExternalOutput")
    hT_o = nc.dram_tensor("hT", [GH, BL, 128], sdt, kind="ExternalOutput")
    cT_o = nc.dram_tensor("cT", [GH, BL, 128], sdt, kind="ExternalOutput")

    # ---- internal: projections, [t, gate, m, b, p] ----
    xh = nc.dram_tensor("xh", [L, 4, GH, BL, 128], f32)

    with tile.TileContext(nc) as tc:
      for _rep in range(repeat):
        # ============ Phase A: projections xh = W_u @ x ============
        with tc.tile_pool(name="proj_in", bufs=1) as pin, \
             tc.tile_pool(name="proj_ps", bufs=4, space="PSUM") as pps, \
             tc.tile_pool(name="proj_ev", bufs=4) as pev, \
             tc.tile_pool(name="proj_c", bufs=1) as pconst:
            xT_s = pin.tile([128, 4, BL * L], f32r)
            nc.sync.dma_start(out=xT_s[:],
                              in_=xT[:, :].rearrange("(k p) n -> p k n", p=128))
            wuT_s = pin.tile([128, 4, 4 * H], f32r)
            nc.sync.dma_start(out=wuT_s[:],
                              in_=wuT[:, :].rearrange("(k p) n -> p k n", p=128))
            bfr_s = pconst.tile([128, 16], f32)
            nc.sync.dma_start(out=bfr_s[:], in_=bfr[:, :])

            for mg in range(32):            # 32 chunks of 128 over 4H
                gate, mH = mg // GH, mg % GH
                for nb in range(BL):        # token chunk = one batch row, 512 t
                    ps = pps.tile([128, 512], f32)
                    for k in range(4):
                        nc.tensor.matmul(
                            ps[:],
                            wuT_s[:, k, mg * 128:(mg + 1) * 128],
                            xT_s[:, k, nb * 512:(nb + 1) * 512],
                            start=(k == 0), stop=(k == 3))
                    ev = pev.tile([128, 512], f32)
                    if gate == 0:
                        bias = bfr_s[:, mH:mH + 1]
                    elif gate == 1:
                        bias = bfr_s[:, 8 + mH:9 + mH]
                    else:
                        bias = None
                    if bias is None:
                        nc.scalar.activation(
                            out=ev[:], in_=ps[:],
                            func=mybir.ActivationFunctionType.Copy)
                    else:
                        nc.scalar.activation(
                            out=ev[:], in_=ps[:],
                            func=mybir.ActivationFunctionType.Identity,
                            bias=bias)
                    # xh[t, gate, mH, nb, p] <- ev[p, t]
                    nc.sync.dma_start(
                        out=xh[:, gate, mH, nb, :].rearrange("t p -> p t"),
                        in_=ev[:])

        # ============ Phase B: recurrence ============
        with tc.tile_pool(name="rec_w", bufs=1) as rw, \
             tc.tile_pool(name="rec_c", bufs=1) as rc, \
             tc.tile_pool(name="rec_x", bufs=4) as rx, \
             tc.tile_pool(name="rec_t", bufs=3) as rt, \
             tc.tile_pool(name="rec_ps", bufs=2, space="PSUM") as rps:
            W = rw.tile([128, 16, H], wdt)
            nc.sync.dma_start(out=W[:],
                              in_=wT[:, :].rearrange("(k p) m -> p k m", p=128))
            vf_s = rc.tile([128, 64], f32)
            nc.sync.dma_start(out=vf_s[:], in_=vf_t[:, :])
            vr_s = rc.tile([128, 64], f32)
            nc.sync.dma_start(out=vr_s[:], in_=vr_t[:, :])
            bo_s = rc.tile([128, 64], f32)
            nc.sync.dma_start(out=bo_s[:], in_=bo_t[:, :])

            # persistent state [c | h]; f32r so tanh's output is legal
            # f32r-matmul input (walrus rounds at write)
            S = rc.tile([128, 128], sdt)
            if rec_dtype == "bf16":
                nc.vector.memset(S[:], 0.0)
            else:
                z0 = rc.tile([128, 128], f32)
                nc.vector.memset(z0[:], 0.0)
                nc.vector.tensor_copy(S[:], z0[:])

            for t in range(l_steps):
                X1 = rx.tile([128, 64], f32, tag="x1")
                nc.sync.dma_start(
                    out=X1[:], in_=xh[t, 0].rearrange("m b p -> p (m b)"))
                X2 = rx.tile([128, 64], f32, tag="x2")
                nc.sync.dma_start(
                    out=X2[:], in_=xh[t, 1].rearrange("m b p -> p (m b)"))
                X34 = rx.tile([128, 128], f32, tag="x34")
                nc.sync.dma_start(
                    out=X34[:],
                    in_=xh[t, 2:4].rearrange("g m b p -> p (g m b)"))

                # matmul z = W_comb @ [c; h]
                if rec_dtype == "bf16":
                    Sb = rt.tile([128, 128], bf16, tag="sb")
                    nc.vector.tensor_copy(Sb[:], S[:])
                else:
                    Sb = S
                z = rps.tile([128, 64], f32)
                for m in range(GH):
                    wm = W[:, :, m * 128:(m + 1) * 128]
                    for k in range(16):
                        nc.tensor.matmul(
                            z[:, m * 8:(m + 1) * 8],
                            wm[:, k, :],
                            Sb[:, k * 8:(k + 1) * 8],
                            start=(k == 0), stop=(k == 15))

                # gates f, r (pre-acts already contain b_f/b_r via projection)
                t1 = rt.tile([128, 64], f32, tag="t1")
                nc.vector.tensor_mul(t1[:], S[:, 0:64], vf_s[:])
                t2 = rt.tile([128, 64], f32, tag="t2")
                nc.vector.tensor_add(t2[:], t1[:], X1[:])
                FR = rt.tile([128, 128], f32, tag="fr")
                nc.scalar.activation(out=FR[:, 0:64], in_=t2[:],
                                     func=mybir.ActivationFunctionType.Sigmoid)
                t3 = rt.tile([128, 64], f32, tag="t3")
                nc.vector.tensor_mul(t3[:], S[:, 0:64], vr_s[:])
                t4 = rt.tile([128, 64], f32, tag="t4")
                nc.vector.tensor_add(t4[:], t3[:], X2[:])
                nc.scalar.activation(out=FR[:, 64:128], in_=t4[:],
                                     func=mybir.ActivationFunctionType.Sigmoid)

                # o gate
                t5 = rt.tile([128, 64], f32, tag="t5")
                nc.vector.tensor_add(t5[:], z[:], bo_s[:])
                O = rt.tile([128, 64], f32, tag="o")
                nc.scalar.activation(out=O[:], in_=t5[:],
                                     func=mybir.ActivationFunctionType.Sigmoid)

                # state update
                u1 = rt.tile([128, 128], f32, tag="u1")
                nc.vector.tensor_sub(u1[:], S[:], X34[:])
                u2 = rt.tile([128, 128], f32, tag="u2")
                nc.vector.tensor_mul(u2[:], FR[:], u1[:])
                u3 = rt.tile([128, 128], f32, tag="u3")
                nc.vector.tensor_add(u3[:], u2[:], X34[:])
                # u4 = [o|o] * [h|c]
                O2 = bass.AP(tensor=O.tensor, offset=O.offset,
                             ap=[O.ap[0], [0, 2], [1, 64]])
                Sswap = bass.AP(tensor=S.tensor, offset=S.offset + 64,
                                ap=[S.ap[0], [-64, 2], [1, 64]])
                u4 = rt.tile([128, 128], f32, tag="u4")
                nc.vector.tensor_mul(u4[:].rearrange("p (g f) -> p g f", g=2),
                                     O2, Sswap)
                u5 = rt.tile([128, 128], f32, tag="u5")
                nc.vector.tensor_add(u5[:], u3[:], u4[:])
                nc.scalar.activation(out=S[:], in_=u5[:],
                                     func=mybir.ActivationFunctionType.Tanh)

                # write h_t to outs[t, m, b, p]
                nc.sync.dma_start(
                    out=outs[t].rearrange("m b p -> p m b"),
                    in_=S[:, 64:128].rearrange("p (m b) -> p m b", m=GH))

            nc.sync.dma_start(
                out=hT_o[:, :, :].rearrange("m b p -> p m b"),
                in_=S[:, 64:128].rearrange("p (m b) -> p m b", m=GH))
            nc.sync.dma_start(
                out=cT_o[:, :, :].rearrange("m b p -> p m b"),
                in_=S[:, 0:64].rearrange("p (m b) -> p m b", m=GH))

    nc.compile()
    return nc


def _get_nc(l_steps: int, rec_dtype: str, repeat: int = 1):
    key = (l_steps, rec_dtype, repeat)
    if key not in _CACHE:
        _CACHE[key] = _build_nc(l_steps, rec_dtype, repeat)
    return _CACHE[key]


def _vec_tile(v: np.ndarray) -> np.ndarray:
    # [1024] -> [128, 64] with (p, m*8+b) = v[m*128+p]
    t = v.reshape(GH, 128).T  # [128, 8]
    return np.repeat(t[:, :, None], BL, axis=2).reshape(128, GH * BL).copy()


def kernel(x, W_u, W_c, W_h, b_f, v_f, b_r, v_r, b_o):
    x = np.asarray(x, np.float32)
    W_u = np.asarray(W_u, np.float32)
    W_c = np.asarray(W_c, np.float32)
    W_h = np.asarray(W_h, np.float32)
    b_f = np.asarray(b_f, np.float32)
    v_f = np.asarray(v_f, np.float32)
    b_r = np.asarray(b_r, np.float32)
    v_r = np.asarray(v_r, np.float32)
    b_o = np.asarray(b_o, np.float32)

    nc = _get_nc(L_STEPS, REC_DTYPE, REPEAT)

    wuT = np.ascontiguousarray(W_u.T)                       # [512, 4096]
    wT = np.concatenate([W_c.T, W_h.T], axis=0)             # [2048, 1024]
    if REC_DTYPE == "bf16":
        wT = wT.astype(ml_dtypes.bfloat16)
    else:
        wT = np.ascontiguousarray(wT)
    vf_t, vr_t, bo_t = _vec_tile(v_f), _vec_tile(v_r), _vec_tile(b_o)
    bfr = np.concatenate([b_f.reshape(GH, 128).T,
                          b_r.reshape(GH, 128).T], axis=1).copy()  # [128,16]

    in_maps = []
    for c in range(NCORES):
        xs = x[c * BL:(c + 1) * BL].reshape(BL * L, I)
        in_maps.append({
            "xT": np.ascontiguousarray(xs.T),
            "wuT": wuT, "wT": wT,
            "vf_t": vf_t, "vr_t": vr_t, "bo_t": bo_t, "bfr": bfr,
        })

    res = bass_utils.run_bass_kernel_spmd(nc, in_maps,
                                          core_ids=list(range(NCORES)))
    # per-core outs [L, GH, BL, 128] -> [BL, L, H]
    outs = np.concatenate(
        [res.results[c]["outs"].transpose(2, 0, 1, 3).reshape(BL, L_STEPS, H)
         for c in range(NCORES)], 0)
    hT = np.concatenate(
        [res.results[c]["hT"].transpose(1, 0, 2).reshape(BL, H)
         for c in range(NCORES)], 0)
    cT = np.concatenate(
        [res.results[c]["cT"].transpose(1, 0, 2).reshape(BL, H)
         for c in range(NCORES)], 0)
    return outs, hT, cT


# revision 21
# speedup vs baseline: 1.0635x; 1.0635x over previous
"""BPTRU cell kernel for Trainium2, 8 NeuronCores, batch-sharded.

Reference computation (per batch row b, sequence length L):
  xh = x @ W_u.T                                   # [L, 4H] projections
  per step: x1,x2,x3,x4 = split(xh[t])
    f = sigmoid(x1 + v_f*c + b_f)
    r = sigmoid(x2 + v_r*c + b_r)
    o = sigmoid(c @ W_c.T + h @ W_h.T + b_o)
    c' = tanh(f*c + (1-f)*x3 + o*h)
    h' = tanh(r*h + (1-r)*x4 + o*c)
  returns (outs [B,L,H], h_T, c_T)

Sharding: batch 64 -> 8 per core, weights replicated (no cross-core comm).
Per-core layouts:
  - projection: out = W_u.T.T @ x.T via PE, float32r (TF32-ish), results
    stored to DRAM as [t, gate, m, b, p] so per-step loads are contiguous.
  - recurrence state S [128, 128] fp32: cols 0:64 = c, 64:128 = h, packed
    (p, m*8+b) with H index = m*128+p.  Matmul weight-stationary bf16.
"""

import os
import numpy as np
import ml_dtypes

import concourse.bass as bass
import concourse.mybir as mybir
import concourse.tile as tile
from concourse import bacc, bass_utils

B, L, I, H = 64, 512, 512, 1024
NCORES = 8
BL = B // NCORES  # 8 batch rows per core
GH = H // 128     # 8 h-chunks of 128

L_STEPS = int(os.environ.get("BPTRU_L", L))
REC_DTYPE = os.environ.get("BPTRU_REC_DTYPE", "f32r")  # bf16 | f32r | fp32
PROJ_DTYPE = os.environ.get("BPTRU_PROJ_DTYPE", "f32r")  # f32r | fp32
REPEAT = int(os.environ.get("BPTRU_REPEAT", 1))

_CACHE = {}


def _build_nc(l_steps: int, rec_dtype: str, repeat: int = 1,
              proj_dtype: str = "f32r"):
    nc = bacc.Bacc("TRN2", target_bir_lowering=False, debug=False,
                   num_devices=NCORES)
    f32 = mybir.dt.float32
    f32r = mybir.dt.float32r
    bf16 = mybir.dt.bfloat16
    wdt = {"bf16": bf16, "f32r": f32r, "fp32": f32}[rec_dtype]
    sdt = f32r if rec_dtype == "f32r" else f32
    pdt = f32r if proj_dtype == "f32r" else f32

    # ---- inputs (per-core) ----
    xT = nc.dram_tensor("xT", [I, BL * L], pdt, kind="ExternalInput")
    wuT = nc.dram_tensor("wuT", [I, 4 * H], pdt, kind="ExternalInput")
    # [W_c.T ; W_h.T] stacked -> lhsT for the recurrence matmul
    wT = nc.dram_tensor("wT", [2 * H, H], wdt, kind="ExternalInput")
    vf_t = nc.dram_tensor("vf_t", [128, 64], f32, kind="ExternalInput")
    vr_t = nc.dram_tensor("vr_t", [128, 64], f32, kind="ExternalInput")
    bo_t = nc.dram_tensor("bo_t", [128, 64], f32, kind="ExternalInput")
    bfr = nc.dram_tensor("bfr", [128, 16], f32, kind="ExternalInput")

    # ---- outputs (per-core); outs layout [t, m, b, p], host transposes ----
    outs = nc.dram_tensor("outs", [l_steps, GH, BL, 128], sdt,
                          kind="ExternalOutput")
    hT_o = nc.dram_tensor("hT", [GH, BL, 128], sdt, kind="ExternalOutput")
    cT_o = nc.dram_tensor("cT", [GH, BL, 128], sdt, kind="ExternalOutput")

    # ---- internal: projections, [t, gate, m, b, p] ----
    xh = nc.dram_tensor("xh", [L, 4, GH, BL, 128], f32)

    with tile.TileContext(nc) as tc:
      for _rep in range(repeat):
        # ============ Phase A: projections xh = W_u @ x ============
        with tc.tile_pool(name="proj_in", bufs=1) as pin, \
             tc.tile_pool(name="proj_ps", bufs=4, space="PSUM") as pps, \
             tc.tile_pool(name="proj_ev", bufs=4) as pev, \
             tc.tile_pool(name="proj_c", bufs=1) as pconst:
            xT_s = pin.tile([128, 4, BL * L], pdt)
            nc.sync.dma_start(out=xT_s[:],
                              in_=xT[:, :].rearrange("(k p) n -> p k n", p=128))
            wuT_s = pin.tile([128, 4, 4 * H], pdt)
            nc.sync.dma_start(out=wuT_s[:],
                              in_=wuT[:, :].rearrange("(k p) n -> p k n", p=128))
            bfr_s = pconst.tile([128, 16], f32)
            nc.sync.dma_start(out=bfr_s[:], in_=bfr[:, :])

            for mg in range(32):            # 32 chunks of 128 over 4H
                gate, mH = mg // GH, mg % GH
                for nb in range(BL):        # token chunk = one batch row, 512 t
                    ps = pps.tile([128, 512], f32)
                    for k in range(4):
                        nc.tensor.matmul(
                            ps[:],
                            wuT_s[:, k, mg * 128:(mg + 1) * 128],
                            xT_s[:, k, nb * 512:(nb + 1) * 512],
                            start=(k == 0), stop=(k == 3))
                    ev = pev.tile([128, 512], f32)
                    if gate == 0:
                        bias = bfr_s[:, mH:mH + 1]
                    elif gate == 1:
                        bias = bfr_s[:, 8 + mH:9 + mH]
                    else:
                        bias = None
                    if bias is None:
                        nc.scalar.activation(
                            out=ev[:], in_=ps[:],
                            func=mybir.ActivationFunctionType.Copy)
                    else:
                        nc.scalar.activation(
                            out=ev[:], in_=ps[:],
                            func=mybir.ActivationFunctionType.Identity,
                            bias=bias)
                    # xh[t, gate, mH, nb, p] <- ev[p, t]
                    nc.sync.dma_start(
                        out=xh[:, gate, mH, nb, :].rearrange("t p -> p t"),
                        in_=ev[:])

        # ============ Phase B: recurrence ============
        with tc.tile_pool(name="rec_w", bufs=1) as rw, \
             tc.tile_pool(name="rec_c", bufs=1) as rc, \
             tc.tile_pool(name="rec_x", bufs=4) as rx, \
             tc.tile_pool(name="rec_t", bufs=3) as rt, \
             tc.tile_pool(name="rec_ps", bufs=2, space="PSUM") as rps:
            W = rw.tile([128, 16, H], wdt)
            nc.sync.dma_start(out=W[:],
                              in_=wT[:, :].rearrange("(k p) m -> p k m", p=128))
            vf_s = rc.tile([128, 64], f32)
            nc.sync.dma_start(out=vf_s[:], in_=vf_t[:, :])
            vr_s = rc.tile([128, 64], f32)
            nc.sync.dma_start(out=vr_s[:], in_=vr_t[:, :])
            bo_s = rc.tile([128, 64], f32)
            nc.sync.dma_start(out=bo_s[:], in_=bo_t[:, :])

            # persistent state [c | h]; f32r so tanh's output is legal
            # f32r-matmul input (walrus rounds at write)
            S = rc.tile([128, 128], sdt)
            if rec_dtype == "f32r":
                z0 = rc.tile([128, 128], f32)
                nc.vector.memset(z0[:], 0.0)
                nc.vector.tensor_copy(S[:], z0[:])
            else:
                nc.vector.memset(S[:], 0.0)

            for t in range(l_steps):
                X1 = rx.tile([128, 64], f32, tag="x1")
                nc.sync.dma_start(
                    out=X1[:], in_=xh[t, 0].rearrange("m b p -> p (m b)"))
                X2 = rx.tile([128, 64], f32, tag="x2")
                nc.sync.dma_start(
                    out=X2[:], in_=xh[t, 1].rearrange("m b p -> p (m b)"))
                X34 = rx.tile([128, 128], f32, tag="x34")
                nc.sync.dma_start(
                    out=X34[:],
                    in_=xh[t, 2:4].rearrange("g m b p -> p (g m b)"))

                # matmul z = W_comb @ [c; h]
                if rec_dtype == "bf16":
                    Sb = rt.tile([128, 128], bf16, tag="sb")
                    nc.vector.tensor_copy(Sb[:], S[:])
                else:
                    Sb = S
                z = rps.tile([128, 64], f32)
                for m in range(GH):
                    wm = W[:, :, m * 128:(m + 1) * 128]
                    for k in range(16):
                        nc.tensor.matmul(
                            z[:, m * 8:(m + 1) * 8],
                            wm[:, k, :],
                            Sb[:, k * 8:(k + 1) * 8],
                            start=(k == 0), stop=(k == 15))

                # gates f, r (pre-acts already contain b_f/b_r via projection)
                t1 = rt.tile([128, 64], f32, tag="t1")
                nc.vector.tensor_mul(t1[:], S[:, 0:64], vf_s[:])
                t2 = rt.tile([128, 64], f32, tag="t2")
                nc.vector.tensor_add(t2[:], t1[:], X1[:])
                FR = rt.tile([128, 128], f32, tag="fr")
                nc.scalar.activation(out=FR[:, 0:64], in_=t2[:],
                                     func=mybir.ActivationFunctionType.Sigmoid)
                t3 = rt.tile([128, 64], f32, tag="t3")
                nc.vector.tensor_mul(t3[:], S[:, 0:64], vr_s[:])
                t4 = rt.tile([128, 64], f32, tag="t4")
                nc.vector.tensor_add(t4[:], t3[:], X2[:])
                nc.scalar.activation(out=FR[:, 64:128], in_=t4[:],
                                     func=mybir.ActivationFunctionType.Sigmoid)

                # o gate
                t5 = rt.tile([128, 64], f32, tag="t5")
                nc.vector.tensor_add(t5[:], z[:], bo_s[:])
                O = rt.tile([128, 64], f32, tag="o")
                nc.scalar.activation(out=O[:], in_=t5[:],
                                     func=mybir.ActivationFunctionType.Sigmoid)

                # state update
                u1 = rt.tile([128, 128], f32, tag="u1")
                nc.vector.tensor_sub(u1[:], S[:], X34[:])
                u2 = rt.tile([128, 128], f32, tag="u2")
                nc.vector.tensor_mul(u2[:], FR[:], u1[:])
                u3 = rt.tile([128, 128], f32, tag="u3")
                nc.vector.tensor_add(u3[:], u2[:], X34[:])
                # u4 = [o|o] * [h|c]
                O2 = bass.AP(tensor=O.tensor, offset=O.offset,
                             ap=[O.ap[0], [0, 2], [1, 64]])
                Sswap = bass.AP(tensor=S.tensor, offset=S.offset + 64,
                                ap=[S.ap[0], [-64, 2], [1, 64]])
                u4 = rt.tile([128, 128], f32, tag="u4")
                nc.vector.tensor_mul(u4[:].rearrange("p (g f) -> p g f", g=2),
                                     O2, Sswap)
                u5 = rt.tile([128, 128], f32, tag="u5")
                nc.vector.tensor_add(u5[:], u3[:], u4[:])
                nc.scalar.activation(out=S[:], in_=u5[:],
                                     func=mybir.ActivationFunctionType.Tanh)

                # write h_t to outs[t, m, b, p]
                nc.sync.dma_start(
                    out=outs[t].rearrange("m b p -> p m b"),
                    in_=S[:, 64:128].rearrange("p (m b) -> p m b", m=GH))

            nc.sync.dma_start(
                out=hT_o[:, :, :].rearrange("m b p -> p m b"),
                in_=S[:, 64:128].rearrange("p (m b) -> p m b", m=GH))
            nc.sync.dma_start(
                out=cT_o[:, :, :].rearrange("m b p -> p m b"),
                in_=S[:, 0:64].rearrange("p (m b) -> p m b", m=GH))

    nc.compile()
    return nc


def _get_nc(l_steps: int, rec_dtype: str, repeat: int = 1,
            proj_dtype: str = "f32r"):
    key = (l_steps, rec_dtype, repeat, proj_dtype)
    if key not in _CACHE:
        _CACHE[key] = _build_nc(l_steps, rec_dtype, repeat, proj_dtype)
    return _CACHE[key]


def _vec_tile(v: np.ndarray) -> np.ndarray:
    # [1024] -> [128, 64] with (p, m*8+b) = v[m*128+p]
    t = v.reshape(GH, 128).T  # [128, 8]
    return np.repeat(t[:, :, None], BL, axis=2).reshape(128, GH * BL).copy()


def kernel(x, W_u, W_c, W_h, b_f, v_f, b_r, v_r, b_o):
    x = np.asarray(x, np.float32)
    W_u = np.asarray(W_u, np.float32)
    W_c = np.asarray(W_c, np.float32)
    W_h = np.asarray(W_h, np.float32)
    b_f = np.asarray(b_f, np.float32)
    v_f = np.asarray(v_f, np.float32)
    b_r = np.asarray(b_r, np.float32)
    v_r = np.asarray(v_r, np.float32)
    b_o = np.asarray(b_o, np.float32)

    nc = _get_nc(L_STEPS, REC_DTYPE, REPEAT, PROJ_DTYPE)

    wuT = np.ascontiguousarray(W_u.T)                       # [512, 4096]
    wT = np.concatenate([W_c.T, W_h.T], axis=0)             # [2048, 1024]
    if REC_DTYPE == "bf16":
        wT = wT.astype(ml_dtypes.bfloat16)
    else:
        wT = np.ascontiguousarray(wT)
    vf_t, vr_t, bo_t = _vec_tile(v_f), _vec_tile(v_r), _vec_tile(b_o)
    bfr = np.concatenate([b_f.reshape(GH, 128).T,
                          b_r.reshape(GH, 128).T], axis=1).copy()  # [128,16]

    in_maps = []
    for c in range(NCORES):
        xs = x[c * BL:(c + 1) * BL].reshape(BL * L, I)
        in_maps.append({
            "xT": np.ascontiguousarray(xs.T),
            "wuT": wuT, "wT": wT,
            "vf_t": vf_t, "vr_t": vr_t, "bo_t": bo_t, "bfr": bfr,
        })

    res = bass_utils.run_bass_kernel_spmd(nc, in_maps,
                                          core_ids=list(range(NCORES)))
    # per-core outs [L, GH, BL, 128] -> [BL, L, H]
    outs = np.concatenate(
        [res.results[c]["outs"].transpose(2, 0, 1, 3).reshape(BL, L_STEPS, H)
         for c in range(NCORES)], 0)
    hT = np.concatenate(
        [res.results[c]["hT"].transpose(1, 0, 2).reshape(BL, H)
         for c in range(NCORES)], 0)
    cT = np.concatenate(
        [res.results[c]["cT"].transpose(1, 0, 2).reshape(BL, H)
         for c in range(NCORES)], 0)
    return outs, hT, cT
